# revision 7
# baseline (speedup 1.0000x reference)
"""CascadedAttentionCell Trainium2 kernel.

Full shapes: inputs [64, 512, 1024] f32, prev_state [64, 1024] f32,
Wa [1024,1024], Ua [1024,1024], Va [1024,1], Ba [1,1024].
Output: context vector [64, 1024] f32.

Sharding: data-parallel over batch across 8 NeuronCores (8 batches/core);
weights replicated.

Per-core plan (B=8 local batches, T=512, D=1024, OUT=1024, P=128):
 - prep: Ua -> SBUF fp16; Wa -> SBUF f32; prev_state transposed via PE;
   WaS^T = Wa^T @ prev^T (fp32 matmuls, N=8); + Ba^T via fused ACT bias-add.
 - per batch: inputs[b] --gpsimd cast-DMA--> SBUF fp16 natural [T,D];
   -> DRAM fp16 scratch -> XBAR dma transpose -> X^T fp16 [D,T].
   S^T[mc] = sum_dc Ua^T[dc,mc] @ X^T[dc]  (fp16 matmuls, N=512, psum f32)
   tanh fused on ACT with per-partition bias (WaS+Ba)^T -> S^T fp16 in SBUF.
   z = sum_mc Va^T[mc] @ S^T[mc]  (fp16, M=1) -> relu on ACT -> zall[b].
 - softmax over T on zall [8,512] (DVE+ACT), cast sm to fp16.
 - sm^T via PE transpose; ctx[b] = sum_tc sm^T[tc,b] @ X_nat[tc] (fp16, M=1).
"""

import numpy as np

import concourse.bass as bass
import concourse.tile as tile
import concourse.mybir as mybir
from concourse import bacc
from concourse.bass import ts
from concourse.bass_utils import run_bass_kernel_spmd
from concourse.masks import make_identity

f32 = mybir.dt.float32
f16 = mybir.dt.float16

N_CORES = 8
B = 8          # batches per core
T = 512
D = 1024
OUT = 1024
P = 128
DC = D // P    # 8 contraction chunks
MC = OUT // P  # 8 out-tile chunks
TC = T // P    # 4 t chunks
NS = 512       # matmul free-dim slice


def build_bass():
    nc = bacc.Bacc("TRN2", target_bir_lowering=False, debug=False,
                   num_devices=N_CORES)

    inputs = nc.dram_tensor("inputs", [B, T, D], f32, kind="ExternalInput").ap()
    prev = nc.dram_tensor("prev_state", [B, OUT], f32, kind="ExternalInput").ap()
    Wa = nc.dram_tensor("Wa", [OUT, OUT], f32, kind="ExternalInput").ap()
    Ua = nc.dram_tensor("Ua", [D, OUT], f32, kind="ExternalInput").ap()
    Va = nc.dram_tensor("Va", [OUT, 1], f32, kind="ExternalInput").ap()
    Ba = nc.dram_tensor("Ba", [1, OUT], f32, kind="ExternalInput").ap()
    out = nc.dram_tensor("out", [B, D], f32, kind="ExternalOutput").ap()

    with tile.TileContext(nc) as tc:
        with (
            tc.tile_pool(name="const", bufs=1) as const,
            tc.tile_pool(name="work", bufs=3) as work,
            tc.tile_pool(name="nat", bufs=B) as natp,
            tc.tile_pool(name="ps_big", bufs=4, space="PSUM") as ps_big,
            tc.tile_pool(name="ps_small", bufs=3, space="PSUM") as ps_small,
            tc.tile_pool(name="dram", bufs=3, space="DRAM") as dram,
        ):
            # ------- input pipelines for the first batches start ASAP -------
            nat16_tiles = []
            nat_dram_tiles = []
            xt_tiles = {}

            def start_input_chain(b):
                nat16 = natp.tile([P, TC, D], f16, tag="nat16")
                nat16_tiles.append(nat16)
                nc.gpsimd.dma_start(
                    nat16[:], inputs[b].rearrange("(c p) d -> p c d", p=P))
                nat_dram = dram.tile([T, D], f16, tag="natdram")
                nat_dram_tiles.append(nat_dram)
                nc.sync.dma_start(
                    nat_dram.rearrange("(c p) d -> p c d", p=P), nat16[:])
                xt = work.tile([P, DC, T], f16, tag="xt")
                xt_tiles[b] = xt
                nc.sync.dma_start_transpose(xt[:], nat_dram[:])

            start_input_chain(0)

            # ---------------- prep ----------------
            ident = const.tile([P, P], f32)
            make_identity(nc, ident)
            ident16 = const.tile([P, P], f16)
            make_identity(nc, ident16)

            # Ua as fp16 lhsT: [p, dc, OUT]
            Ua_sb = const.tile([P, DC, OUT], f16)
            nc.gpsimd.dma_start(Ua_sb[:], Ua.rearrange("(c p) o -> p c o", p=P))

            start_input_chain(1)

            # Wa fp32: [p, oc, OUT]
            Wa_sb = const.tile([P, MC, OUT], f32)
            nc.sync.dma_start(Wa_sb[:], Wa.rearrange("(c p) o -> p c o", p=P))

            # Va fp16 gather: [p, mc]
            Va_sb = const.tile([P, MC], f16)
            nc.gpsimd.dma_start(Va_sb[:], Va.rearrange("(c p) one -> p (c one)", p=P))

            # Ba^T fp32 gather: [p, mc]
            BaT_sb = const.tile([P, MC], f32)
            nc.sync.dma_start(BaT_sb[:], Ba.rearrange("one (c p) -> p (one c)", p=P))

            # prev_state [8, 1024] -> prevT [p, oc, b]
            prev_sb = const.tile([B, OUT], f32)
            nc.sync.dma_start(prev_sb[:], prev[:])
            prevT_sb = const.tile([P, MC, B], f32)
            for oc in range(MC):
                pt_ps = ps_small.tile([P, B], f32, tag="psm")
                nc.tensor.transpose(pt_ps[:], prev_sb[:, ts(oc, P)], ident[:B, :B])
                nc.scalar.copy(prevT_sb[:, oc, :], pt_ps[:])

            # WaS^T + Ba^T: [p, mc, b]
            WaSBaT_sb = const.tile([P, MC, B], f32)
            for mc in range(MC):
                was_ps = ps_small.tile([P, B], f32, tag="psm")
                for oc in range(MC):
                    nc.tensor.matmul(was_ps[:], Wa_sb[:, oc, ts(mc, P)],
                                     prevT_sb[:, oc, :],
                                     start=(oc == 0), stop=(oc == MC - 1))
                nc.scalar.activation(WaSBaT_sb[:, mc, :], was_ps[:],
                                     mybir.ActivationFunctionType.Identity,
                                     bias=BaT_sb[:, mc:mc + 1], scale=1.0)

            zall = const.tile([B, T], f32)

            # ---------------- per-batch phase 1 ----------------
            for b in range(B):
                if b + 2 <= B - 1:
                    start_input_chain(b + 2)
                xt = xt_tiles[b]

                # S^T tiles + fused tanh/bias; z matmuls
                st = work.tile([P, MC, T], f16, tag="st")
                for mc in range(MC):
                    st_ps = ps_big.tile([P, NS], f32, tag="stps")
                    for dc in range(DC):
                        nc.tensor.matmul(st_ps[:], Ua_sb[:, dc, ts(mc, P)],
                                         xt[:, dc, :],
                                         start=(dc == 0), stop=(dc == DC - 1))
                    nc.scalar.activation(st[:, mc, :], st_ps[:],
                                         mybir.ActivationFunctionType.Tanh,
                                         bias=WaSBaT_sb[:, mc, b:b + 1], scale=1.0)

                z_ps = ps_small.tile([1, T], f32, tag="psm")
                for mc in range(MC):
                    nc.tensor.matmul(z_ps[:], Va_sb[:, mc:mc + 1], st[:, mc, :],
                                     start=(mc == 0), stop=(mc == MC - 1))
                z_sb = work.tile([1, T], f32, tag="zsb")
                nc.scalar.activation(z_sb[:], z_ps[:],
                                     mybir.ActivationFunctionType.Relu)
                nc.sync.dma_start(zall[b:b + 1, :], z_sb[:])

            # ---------------- softmax over T on [B, T] ----------------
            negmax = const.tile([B, 1], f32)
            nc.vector.reduce_max(negmax[:], zall[:], axis=mybir.AxisListType.X,
                                 negate=True)
            esb = const.tile([B, T], f32)
            nc.scalar.activation(esb[:], zall[:],
                                 mybir.ActivationFunctionType.Exp,
                                 bias=negmax[:], scale=1.0)
            ssum = const.tile([B, 1], f32)
            nc.vector.reduce_sum(ssum[:], esb[:], axis=mybir.AxisListType.X)
            rsum = const.tile([B, 1], f32)
            nc.vector.reciprocal(rsum[:], ssum[:])
            sm16 = const.tile([B, T], f16)
            nc.vector.tensor_scalar_mul(sm16[:], esb[:], rsum[:])

            # sm^T: [p, tc, b] fp16
            smT = const.tile([P, TC, B], f16)
            for tcI in range(TC):
                smt_ps = ps_small.tile([P, B], f16, tag="psm")
                nc.tensor.transpose(smt_ps[:], sm16[:, ts(tcI, P)], ident16[:B, :B])
                nc.scalar.copy(smT[:, tcI, :], smt_ps[:])

            # ---------------- ctx matmuls ----------------
            for b in range(B):
                nat16 = nat16_tiles[b]
                ctx_sb = work.tile([1, D], f32, tag="ctx")
                for n in range(D // NS):
                    ctx_ps = ps_small.tile([1, NS], f32, tag="psm")
                    for tcI in range(TC):
                        nc.tensor.matmul(ctx_ps[:], smT[:, tcI, b:b + 1],
                                         nat16[:, tcI, ts(n, NS)],
                                         start=(tcI == 0), stop=(tcI == TC - 1))
                    nc.scalar.copy(ctx_sb[:, ts(n, NS)], ctx_ps[:])
                nc.sync.dma_start(out[b:b + 1, :], ctx_sb[:])

    nc.compile()
    return nc


_NC = None


def _get_nc():
    global _NC
    if _NC is None:
        _NC = build_bass()
    return _NC


def run(inputs, prev_state, Wa, Ua, Va, Ba, **spmd_kwargs):
    nc = _get_nc()
    inputs = np.ascontiguousarray(inputs, dtype=np.float32)
    prev_state = np.ascontiguousarray(prev_state, dtype=np.float32)
    weights = {
        "Wa": np.ascontiguousarray(Wa, dtype=np.float32),
        "Ua": np.ascontiguousarray(Ua, dtype=np.float32),
        "Va": np.ascontiguousarray(Va, dtype=np.float32),
        "Ba": np.ascontiguousarray(Ba, dtype=np.float32),
    }
    in_maps = []
    for c in range(N_CORES):
        sl = slice(c * B, (c + 1) * B)
        in_maps.append({
            "inputs": inputs[sl],
            "prev_state": prev_state[sl],
            **weights,
        })
    return run_bass_kernel_spmd(nc, in_maps, core_ids=list(range(N_CORES)),
                                **spmd_kwargs)


def kernel(inputs, prev_state, Wa, Ua, Va, Ba):
    res = run(inputs, prev_state, Wa, Ua, Va, Ba)
    return np.concatenate([r["out"] for r in res.results], axis=0)
